# revision 37
# baseline (speedup 1.0000x reference)
"""AdapCNN block on 8 TRN2 NeuronCores (raw Bass, hand-rolled semaphores).

Strategy (data-parallel over batch, 2 samples per core):
  - The tiny FMN weight-generator MLP (0.8% of FLOPs) runs on host in f32;
    the generated per-sample conv weights are sharded along B to the cores
    (the "shard the generated per-sample weights along B" hint).
  - Each core runs the per-sample 64->64 3x3 VALID conv on its 2 samples.

Conv-as-matmul scheme (75% PE utilization, the max for this decomposition):
  SBUF holds a row-pair layout of x: partitions 0:64 = channels at row r,
  partitions 64:128 = channels at row r+1 (prepared host-side so DMAs write
  all 128 partitions at full bandwidth).  One matmul per kw with
  lhsT[(t*64+c), (dq*64+o)] = W[o,c,t+dq,kw] * (0.5 if t+dq==1 else 1)
  computes, for PSUM slot j: partitions 0:64 += (kh0 + kh1/2) of output row j,
  partitions 64:128 += (kh1/2 + kh2) of output row j-1.  Output row q =
  psum[0:64, q] + psum[64:128, q+1] + bias: ACT does the cross-partition copy
  of the upper half to SBUF (the only legal 2-PSUM-operand workaround), DVE
  adds it to the lower half + bias in one scalar_tensor_tensor op.

Pipeline (raw nc.Block per engine, counting semaphores):
  - superblock = 2 PSUM banks (8 slots -> 7 output rows), 18 per sample,
    4 psum slots rotating so matmuls never wait on the eviction chain
  - x arrives as 18 uniform 15-row chunks (per-chunk DMA semaphores: a
    shared counter is unsafe because queue completions interleave)
  - consecutive superblocks fill the two partition halves of a shared
    staging tile; its two output DMAs land on disjoint DMA port groups
  - an 8-matmul zero warm-up flips the PE HAM clock gate to 2.4GHz during
    the fixed ~7us kernel prologue
  - ob staging reuse is guarded by per-slot DMA semaphores; psum/tmp reuse
    by the DVE op counter; Block(no_gpsimd_drain=True) + explicit final
    waits cheapen the kernel tail
  - compute dtype bf16 (PSUM accumulates f32); y written bf16 and
    upconverted to f32 on host (rel err ~2.8e-3 end to end)
"""
import sys

if '/opt/trn_rl_repo' not in sys.path:
    sys.path.insert(0, '/opt/trn_rl_repo')

import numpy as np
import ml_dtypes

B, CIN, COUT, K = 16, 64, 64, 3
H = W = 128
OH = OW = 126
FC, FMN0, FMN1, G = 512, 512, 512, 4
CNN_PARA = CIN * COUT * K * K + COUT
NCORES = 8
NS = B // NCORES          # samples per core
XROWS = 127               # row-pair layout rows per sample
SB = 7                    # output rows per superblock (8 slots, 2 banks)
NSB = OH // SB            # 18 superblocks per sample

_cached = {}


def _build_module():
    import concourse.mybir as mybir
    from concourse import bacc

    f32 = mybir.dt.float32
    bf16 = mybir.dt.bfloat16

    nc = bacc.Bacc("TRN2", target_bir_lowering=False, debug=False,
                   num_devices=NCORES)
    x_ext = nc.declare_dram_parameter("xh", [NS, 128, XROWS, W], bf16,
                                      isOutput=False)
    wt_ext = nc.declare_dram_parameter("wt", [128, NS * 3 * 128], bf16,
                                       isOutput=False)
    b_ext = nc.declare_dram_parameter("bias", [COUT, NS], f32, isOutput=False)
    y_ext = nc.declare_dram_parameter("y", [NS, COUT, OH, OW], bf16,
                                      isOutput=True)

    add = mybir.AluOpType.add

    # static SBUF; x arrives as 18 uniform 15-row chunks (chunk c of sample s
    # feeds superblocks 2c and 2c+1, rows [14c, 14c+15))
    NCH = 9
    wz = nc.alloc_sbuf_tensor("wz", [128, 512], bf16).ap()
    wt_sb = nc.alloc_sbuf_tensor("wt_sb", [128, NS, 3, 128], bf16).ap()
    bias_sb = nc.alloc_sbuf_tensor("bias_sb", [COUT, NS], f32).ap()
    xbs = [[nc.alloc_sbuf_tensor(f"xb{s0}_{c}", [128, 15, W], bf16).ap()
            for c in range(NCH)] for s0 in range(NS)]
    tmps = [nc.alloc_sbuf_tensor(f"tmp{j}", [64, SB, OW], f32).ap()
            for j in range(4)]
    obs = [nc.alloc_sbuf_tensor(f"ob{j}", [128, SB, OW], bf16).ap()
           for j in range(4)]
    pss = [nc.alloc_psum_tensor(f"ps{j}", [128, SB + 1, 128], f32).ap()
           for j in range(4)]

    NSBT = NS * NSB                                # 36 superblocks total
    NPAIR = NSBT // 2                              # 18 ob pairs

    import contextlib
    with contextlib.ExitStack() as ctx:
        s_xc = [ctx.enter_context(nc.semaphore(f"s_xc{i}"))
                for i in range(NS * NCH)]
        s_wt = ctx.enter_context(nc.semaphore("s_wt"))
        s_b = ctx.enter_context(nc.semaphore("s_b"))
        s_ob = [ctx.enter_context(nc.semaphore(f"s_ob{j}")) for j in range(4)]
        s_mm = ctx.enter_context(nc.semaphore("s_mm"))
        s_act = ctx.enter_context(nc.semaphore("s_act"))
        s_dve = ctx.enter_context(nc.semaphore("s_dve"))
        s_ws = ctx.enter_context(nc.semaphore("s_ws"))
        block = ctx.enter_context(nc.Block(no_gpsimd_drain=True))

        @block.sync
        def _(sy):
            def xdma(s0, c):
                sy.dma_start(
                    xbs[s0][c][:], x_ext[s0, :, 14 * c:14 * c + 15, :]
                ).then_inc(s_xc[s0 * NCH + c], 16)
            xdma(0, 0)
            sy.dma_start(wt_sb.rearrange("p s k m -> p (s k m)"),
                         wt_ext[:]).then_inc(s_wt, 16)
            for c in range(1, NCH):
                xdma(0, c)
            for c in range(NCH):
                xdma(1, c)

        @block.gpsimd
        def _(g):
            g.memset(wz[:], 0.0).then_inc(s_ws, 1)
            g.dma_start(bias_sb[:], b_ext[:]).then_inc(s_b, 16)
            for p in range(NPAIR):
                i1 = 2 * p + 1                     # odd superblock of pair
                s0, b1 = i1 // NSB, i1 % NSB
                r0 = SB * (b1 - 1)
                g.wait_ge(s_dve, 2 * p + 2)
                ob = obs[p % 4]
                g.dma_start(y_ext[s0, :, r0:r0 + SB, :],
                            ob[0:64, :, :]).then_inc(s_ob[p % 4], 16)
                g.dma_start(y_ext[s0, :, r0 + SB:r0 + 2 * SB, :],
                            ob[64:128, :, :]).then_inc(s_ob[p % 4], 16)
            for j in range(4):
                users = len(range(j, NPAIR, 4))
                g.wait_ge(s_ob[j], 32 * users)

        @block.tensor
        def _(t):
            t.wait_ge(s_ws, 1)
            for _ in range(8):
                nc.tensor.matmul(pss[0][:, 0:4, 0:OW], wz[:, 0:128],
                                 wz[:, 0:504], start=True, stop=True)
            t.wait_ge(s_wt, 16)
            waited = set()
            for i in range(NSBT):
                s0, bix = i // NSB, i % NSB
                c = s0 * NCH + bix // 2
                if c not in waited:
                    t.wait_ge(s_xc[c], 16)
                    waited.add(c)
                if i >= 4:
                    t.wait_ge(s_dve, i - 3)
                lj0 = SB * (bix % 2)
                xb = xbs[s0][bix // 2]
                ps = pss[i % 4]
                for kw in range(3):
                    for kb in range(2):
                        ss = 4 * kb
                        mm = nc.tensor.matmul(
                            ps[:, ss:ss + 4, 0:OW],
                            wt_sb[:, s0, kw, :],
                            xb[:, lj0 + ss:lj0 + ss + 4, kw:kw + OW],
                            start=(kw == 0), stop=(kw == 2))
                        if kw == 2 and kb == 1:
                            mm.then_inc(s_mm, 1)

        @block.scalar
        def _(sc):
            for i in range(NSBT):
                sc.wait_ge(s_mm, i + 1)
                if i >= 4:
                    sc.wait_ge(s_dve, i - 3)
                nc.scalar.copy(
                    tmps[i % 4][:],
                    pss[i % 4][64:128, 1:1 + SB, 0:OW]).then_inc(s_act, 1)

        @block.vector
        def _(v):
            v.wait_ge(s_b, 16)
            for i in range(NSBT):
                s0 = i // NSB
                p, dq = i // 2, i % 2
                v.wait_ge(s_act, i + 1)
                if dq == 0 and p >= 4:
                    v.wait_ge(s_ob[p % 4], 32 * (p // 4))
                nc.vector.scalar_tensor_tensor(
                    obs[p % 4][64 * dq:64 * dq + 64, :, :],
                    pss[i % 4][0:64, 0:SB, 0:OW],
                    bias_sb[:, s0:s0 + 1],
                    tmps[i % 4][:],
                    add, add).then_inc(s_dve, 1)

    nc.compile()
    return nc


def _fmn_host(fc_in, w1, b1, w2, b2, w3, b3):
    h = np.maximum(fc_in @ w1.T + b1, 0.0)
    h = np.maximum(h @ w2.T + b2, 0.0)
    hg = h.reshape(h.shape[0], G, FMN1 // G)
    o = np.einsum('bgi,goi->bgo', hg, w3,
                  dtype=np.float32).reshape(h.shape[0], -1) + b3
    return np.maximum(o, 0.0)


def _prep_inputs(x, fc_in, w1, b1, w2, b2, w3, b3):
    wb = _fmn_host(fc_in, w1, b1, w2, b2, w3, b3)          # [B, CNN_PARA]
    weight = wb[:, :-COUT].reshape(B, COUT, CIN, K, K)
    bias = wb[:, -COUT:]                                   # [B, COUT]

    # lhsT[s, kw, t*64+c, dq*64+o] = weight[s, o, c, t+dq, kw] * scale
    wk = weight.transpose(0, 4, 3, 2, 1)                   # [B, kw, kh, c, o]
    lhsT = np.empty((B, 3, 128, 128), np.float32)
    for t in (0, 1):
        for dq in (0, 1):
            kh = t + dq
            sc = 0.5 if kh == 1 else 1.0
            lhsT[:, :, t * 64:t * 64 + 64, dq * 64:dq * 64 + 64] = \
                wk[:, :, kh] * sc
    lhsT = lhsT.astype(ml_dtypes.bfloat16)
    # device layout: [partition, s, kw, m]
    lhsT = lhsT.transpose(2, 0, 1, 3)                      # [128, B, 3, 128]

    xb = x.astype(ml_dtypes.bfloat16)                      # [B, 64, 128, 128]
    xpair = np.empty((B, 128, XROWS, W), ml_dtypes.bfloat16)
    xpair[:, :64] = xb[:, :, 0:XROWS]
    xpair[:, 64:] = xb[:, :, 1:XROWS + 1]

    in_maps = []
    for c in range(NCORES):
        s0 = NS * c
        in_maps.append({
            "xh": np.ascontiguousarray(xpair[s0:s0 + NS]),
            "wt": np.ascontiguousarray(
                lhsT[:, s0:s0 + NS].reshape(128, NS * 3 * 128)),
            "bias": np.ascontiguousarray(bias[s0:s0 + NS].T),
        })
    return in_maps


def kernel(x, fc_in, w1, b1, w2, b2, w3, b3, splits):
    from concourse.bass_utils import run_bass_kernel_spmd

    x = np.asarray(x, np.float32)
    args = [np.asarray(a, np.float32)
            for a in (fc_in, w1, b1, w2, b2, w3, b3)]
    in_maps = _prep_inputs(x, *args)

    if 'nc' not in _cached:
        _cached['nc'] = _build_module()
    nc = _cached['nc']

    res = run_bass_kernel_spmd(nc, in_maps, core_ids=list(range(NCORES)))

    out = np.empty((B * COUT, OH, OW), np.float32)
    for c in range(NCORES):
        y = res.results[c]["y"]                            # [NS, COUT, OH, OW]
        out[NS * COUT * c:NS * COUT * (c + 1)] = \
            np.asarray(y, np.float32).reshape(NS * COUT, OH, OW)
    return out.reshape(1, B * COUT, 1, OH, OW)
